# revision 41
# baseline (speedup 1.0000x reference)
"""Longformer sliding-window self-attention on 8 Trainium2 NeuronCores.

Problem: hidden [1, 8192, 768] -> QKV projections (768x768 each) ->
12-head sliding-window attention (one-sided window 256) -> ctx [1, 8192, 768].

Sharding: sequence-parallel across 8 cores. Each core owns 1024 query
positions and recomputes K/V projections over its 1024+2*256 halo-extended
slice (host passes the transposed, zero-padded hidden slice per core).

v2 (this file): all matmuls bf16 (hidden/W/q/k/v/probs), which halves
weight-load (LDWEIGHTS) time and input DMA vs the fp32r baseline and keeps
the PE stream dense so the HAM clock gate stays at 8/8. Sequence-boundary
masking is folded into the v' validity column (col 64 per head) and the
zeroed v rows, so only the band-geometry mask remains: a single static
[128, 1024] bf16 mask applied to the 4 of 6 key blocks that intersect the
band edge (score columns are ordered (j0,j1,j4,j5,j2,j3) so one DVE
multiply covers them). Softmax denominators are extracted from PV row 64
by ACT Copy into a [2, 1536] row pair per q-tile and inverted with one
reciprocal_approx_fast, replacing 48 single-partition DVE reciprocals.

Per-core device program:
  - qT [768,1024], kT [768,1536] feature-major bf16 projections.
  - v' [1536, 12*65] bf16 with per-head validity column (softmax
    denominator row); bias+padding via a K=1 matmul against the
    position-validity row.
  - Per (256-query tile x head): scores^T [768k, 256q] in PSUM via 6
    K=64 bf16 matmuls; ACT exp -> bf16 probs; band-mask multiply on the
    first 1024 cols; 6 accumulating bf16 PV matmuls -> ctx' [65, 256].
  - Normalize: ACT-copied denominators -> reciprocal_approx_fast ->
    K=2 selector matmul broadcast -> DVE multiply -> DMA out.
"""
import numpy as np
from contextlib import ExitStack

import ml_dtypes

import concourse.bass as bass
import concourse.bacc as bacc
import concourse.mybir as mybir
from concourse.tile import TileContext
from concourse.bass_utils import run_bass_kernel_spmd

F32 = mybir.dt.float32
F32R = mybir.dt.float32r
BF16 = mybir.dt.bfloat16

NCORES = 8
S, HID, H, D, W = 8192, 768, 12, 64, 256
SL = S // NCORES            # 1024 queries per core
EXT = SL + 2 * W            # 1536 extended positions (with halo)
KB = HID // 128             # 6 feature blocks
NT = SL // 256              # 4 query tiles of 256
NJ = 6                      # key tiles of 128 per query tile
NST = EXT // 128            # 12 sequence tiles for v'
EXPF = mybir.ActivationFunctionType.Exp
COPYF = mybir.ActivationFunctionType.Copy
MUL = mybir.AluOpType.mult
# Compact 1280-col score layout. Key block j0 only matters for the first
# 128 queries of a 256-query tile (band: c <= p < 128) and j5 only for the
# last 128 (c >= p + 128 >= 128), so their valid halves share slot 0:
#   slot0 = [j0 @ q 0:128 | j5 @ q 128:256], slot1 = j1, slot2 = j4,
#   slot3 = j2, slot4 = j3.  Slots 0-2 carry the band mask; 3-4 are fully
#   in-band.  (q0, nq, col) per key block j:
SLOT = {0: (0, 128, 0), 5: (128, 128, 128),
        1: (0, 256, 256), 4: (0, 256, 512),
        2: (0, 256, 768), 3: (0, 256, 1024)}
NSC = 1280                  # score/prob columns per (tile, head)


def _build():
    nc = bacc.Bacc(
        "TRN2",
        target_bir_lowering=False,
        debug=False,
        num_devices=NCORES,
    )
    hT_d = nc.declare_dram_parameter("hT", [HID, EXT], BF16, isOutput=False)
    wq_d = nc.declare_dram_parameter("wq", [HID, HID], BF16, isOutput=False)
    wk_d = nc.declare_dram_parameter("wk", [HID, HID], BF16, isOutput=False)
    wv_d = nc.declare_dram_parameter("wv", [HID, HID], BF16, isOutput=False)
    # biasqk cols 0:12, position-validity (transposed) cols 12:24
    bp_d = nc.declare_dram_parameter("biaspvt", [128, 2 * KB + NST], F32, isOutput=False)
    # validity row 0:EXT, v-bias row EXT:EXT+HID
    pvbv_d = nc.declare_dram_parameter("pvbv", [1, EXT + HID], BF16, isOutput=False)
    sel2_d = nc.declare_dram_parameter("sel2", [1, 256], F32R, isOutput=False)
    out_d = nc.declare_dram_parameter("out", [NT, 128, NJ * 256], F32, isOutput=True)

    with ExitStack() as ctx:
        tc = ctx.enter_context(TileContext(nc))
        pH = ctx.enter_context(tc.tile_pool(name="h", bufs=1))
        pW = ctx.enter_context(tc.tile_pool(name="w", bufs=18))
        pQ = ctx.enter_context(tc.tile_pool(name="q", bufs=1))
        pK = ctx.enter_context(tc.tile_pool(name="k", bufs=1))
        pV = ctx.enter_context(tc.tile_pool(name="v", bufs=1))
        pProb = ctx.enter_context(tc.tile_pool(name="prob", bufs=3))
        pMask = ctx.enter_context(tc.tile_pool(name="mask", bufs=1))
        pOut = ctx.enter_context(tc.tile_pool(name="outp", bufs=2))
        pRr = ctx.enter_context(tc.tile_pool(name="rr", bufs=2))
        pMisc = ctx.enter_context(tc.tile_pool(name="misc", bufs=1))
        pSc = ctx.enter_context(tc.tile_pool(name="scps", bufs=2, space="PSUM"))
        pPs = ctx.enter_context(tc.tile_pool(name="ps", bufs=2, space="PSUM"))

        # ---- weight & hidden DMAs first so projections start ASAP, spread
        # over the three hwdge queues (sync: wq,wk; gpsimd: hT even + wv;
        # scalar: consts + hT odd).  Small constants are batched into two
        # parameters since each DMA costs ~0.6us of queue time regardless
        # of size.
        wq_t = []
        h_t = [None] * KB
        for k in range(KB):
            w = pW.tile([128, HID], BF16, tag="w")
            nc.sync.dma_start(w[:], wq_d[k * 128:(k + 1) * 128, :])
            wq_t.append(w)
            ht = pH.tile([128, EXT], BF16, tag=f"h{k}")
            eng = nc.gpsimd if k % 2 == 0 else nc.scalar
            eng.dma_start(ht[:], hT_d[k * 128:(k + 1) * 128, :])
            h_t[k] = ht
        wk_t = []
        for k in range(KB):
            w = pW.tile([128, HID], BF16, tag="w")
            nc.sync.dma_start(w[:], wk_d[k * 128:(k + 1) * 128, :])
            wk_t.append(w)
        bp_sb = pMisc.tile([128, 2 * KB + NST], F32, tag="biaspvt")
        nc.scalar.dma_start(bp_sb[:], bp_d[:])
        bias_sb = bp_sb[:, 0:2 * KB]
        pvt_sb = bp_sb[:, 2 * KB:]
        pvbv_sb = pMisc.tile([1, EXT + HID], BF16, tag="pvbv")
        nc.scalar.dma_start(pvbv_sb[:], pvbv_d[:])
        pvrow_sb = pvbv_sb[:, 0:EXT]
        bvrow_sb = pvbv_sb[:, EXT:]
        sel2_sb = pMisc.tile([1, 256], F32R, tag="sel2")
        nc.scalar.dma_start(sel2_sb[:], sel2_d[:])

        # static band mask for slots 0-2 of the compact layout:
        #   slot0 lo (j0 @ q 0:128):   valid iff p - c >= 0
        #   slot0 hi (j5 @ q 128:256): valid iff c - p >= 0 (local c)
        #   slot1    (j1 @ q 0:256):   valid iff p + 128 - c >= 0
        #   slot2    (j4 @ q 0:256):   valid iff c - p >= 0
        mk = pMask.tile([128, 3 * 256], BF16, tag="mask", name="mask")
        nc.gpsimd.memset(mk[:], 1.0)
        nc.gpsimd.affine_select(
            out=mk[:, 0:128], in_=mk[:, 0:128],
            compare_op=mybir.AluOpType.is_ge,
            fill=0.0, base=0, pattern=[[-1, 128]], channel_multiplier=1)
        nc.gpsimd.affine_select(
            out=mk[:, 128:256], in_=mk[:, 128:256],
            compare_op=mybir.AluOpType.is_ge,
            fill=0.0, base=0, pattern=[[1, 128]], channel_multiplier=-1)
        nc.gpsimd.affine_select(
            out=mk[:, 256:512], in_=mk[:, 256:512],
            compare_op=mybir.AluOpType.is_ge,
            fill=0.0, base=128, pattern=[[-1, 256]], channel_multiplier=1)
        nc.gpsimd.affine_select(
            out=mk[:, 512:768], in_=mk[:, 512:768],
            compare_op=mybir.AluOpType.is_ge,
            fill=0.0, base=0, pattern=[[1, 256]], channel_multiplier=-1)

        qT_t = [pQ.tile([128, SL], BF16, tag=f"q{m}", name=f"qT{m}") for m in range(KB)]
        kT_t = [pK.tile([128, EXT], BF16, tag=f"k{m}", name=f"kT{m}") for m in range(KB)]
        v_t = [pV.tile([128, H * 65], BF16, tag=f"v{st}", name=f"vp{st}") for st in range(NST)]

        def emit_qproj(c2):
            e0 = W + c2 * 512
            for m in range(KB):
                ps = pPs.tile([128, 512], F32, tag="ps")
                for k in range(KB):
                    nc.tensor.matmul(
                        ps[:], lhsT=wq_t[k][:, m * 128:(m + 1) * 128],
                        rhs=h_t[k][:, e0:e0 + 512],
                        start=(k == 0), stop=(k == KB - 1))
                nc.vector.tensor_scalar_add(
                    qT_t[m][:, c2 * 512:(c2 + 1) * 512], ps[:],
                    bias_sb[:, m:m + 1])

        def emit_qproj_tm(t, m):
            # N=256 q-projection chain for one (q-tile, row-block): rides
            # just-in-time inside the attention of earlier tiles
            e0 = W + t * 256
            ps = pPs.tile([128, 256], F32, tag="ps")
            for k in range(KB):
                nc.tensor.matmul(
                    ps[:], lhsT=wq_t[k][:, m * 128:(m + 1) * 128],
                    rhs=h_t[k][:, e0:e0 + 256],
                    start=(k == 0), stop=(k == KB - 1))
            nc.vector.tensor_scalar_add(
                qT_t[m][:, t * 256:(t + 1) * 256], ps[:],
                bias_sb[:, m:m + 1])

        def emit_kproj_m(c, m):
            ps = pPs.tile([128, 512], F32, tag="ps")
            for k in range(KB):
                nc.tensor.matmul(
                    ps[:], lhsT=wk_t[k][:, m * 128:(m + 1) * 128],
                    rhs=h_t[k][:, c * 512:(c + 1) * 512],
                    start=(k == 0), stop=(k == KB - 1))
            nc.vector.tensor_scalar_add(
                kT_t[m][:, c * 512:(c + 1) * 512], ps[:],
                bias_sb[:, KB + m:KB + m + 1])

        def emit_kproj(c):
            for m in range(KB):
                emit_kproj_m(c, m)

        def emit_vproj(st):
            vt = v_t[st]
            vv = vt[:].rearrange("p (h x) -> p h x", x=65)
            # validity column doubles as the softmax-denominator selector:
            # invalid (zero-padded) key rows contribute to neither ctx nor
            # the denominator, which subsumes the sequence-boundary mask
            dcol = vv[:, :, 64:65]
            pvv = pvt_sb[:, st:st + 1].rearrange("p (a b) -> p a b", a=1)
            _, pvb = bass.broadcast_tensor_aps(dcol, pvv)
            nc.vector.tensor_copy(dcol, pvb)
            for (f0, nf) in ((0, 512), (512, 256)):
                ps = pPs.tile([128, nf], F32, tag="ps")
                for k in range(KB):
                    nc.tensor.matmul(
                        ps[:], lhsT=h_t[k][:, st * 128:(st + 1) * 128],
                        rhs=wv_t[k][:, f0:f0 + nf],
                        start=(k == 0), stop=False)
                nc.tensor.matmul(
                    ps[:], lhsT=pvrow_sb[0:1, st * 128:(st + 1) * 128],
                    rhs=bvrow_sb[0:1, f0:f0 + nf], start=False, stop=True)
                nc.scalar.activation(
                    vv[:, f0 // 64:(f0 + nf) // 64, 0:64],
                    ps[:].rearrange("p (h x) -> p h x", x=64), COPYF)

        def emit_scores(t, h):
            kb, po = h // 2, (h % 2) * 64
            sc = pSc.tile([128, NSC], F32, tag="sc")
            for j in (0, 5, 1, 4, 2, 3):
                q0, nq, cs = SLOT[j]
                k0 = t * 256 + j * 128
                nc.tensor.matmul(
                    sc[:, cs:cs + nq],
                    lhsT=kT_t[kb][po:po + 64, k0:k0 + 128],
                    rhs=qT_t[kb][po:po + 64, t * 256 + q0:t * 256 + q0 + nq],
                    start=True, stop=True)
            pr = pProb.tile([128, NSC], BF16, tag="pr")
            nc.scalar.activation(pr[:], sc[:], EXPF)
            nc.vector.tensor_mul(pr[:, 0:768], pr[:, 0:768], mk[:])
            return pr

        def emit_pv(t, h, prm, ob, rr):
            # j1 (full width) opens the accumulation group — a start on a
            # half-width block would zero the other half's columns
            cx = pPs.tile([65, 256], F32, tag="ps")
            for j in (1, 0, 5, 4, 2, 3):
                q0, nq, cs = SLOT[j]
                nc.tensor.matmul(
                    cx[:, q0:q0 + nq],
                    lhsT=v_t[2 * t + j][:, h * 65:(h + 1) * 65],
                    rhs=prm[:, cs:cs + nq],
                    start=(j == 1), stop=(j == 3),
                    skip_group_check=True)
            # stash unnormalized ctx' and the denominator row; normalization
            # is batched per q-tile so the PE stream stays dense
            c0 = (h // 2) * 256
            nc.vector.tensor_copy(
                ob[(h % 2) * 64:(h % 2) * 64 + 64, c0:c0 + 256], cx[0:64, :])
            nc.vector.tensor_copy(
                rr[h % 2][0:1, c0:c0 + 256], cx[64:65, :])

        def emit_norm_ck(t, ob, rr, c0, c1):
            # broadcast the RAW denominators across partitions with K=1
            # selector matmuls, then invert the wide [128, c1-c0] tile
            # (c1-c0 elems/lane: fast) and multiply — avoids
            # single-partition reciprocals entirely. One chunk at a time so
            # normalization + output DMA pipeline behind the attention.
            nw = c1 - c0
            bc = pPs.tile([128, nw], F32, tag="ps")
            nc.tensor.matmul(
                bc[:], lhsT=sel2_sb[0:1, 0:128],
                rhs=rr[0][0:1, c0:c1],
                start=True, stop=False)
            nc.tensor.matmul(
                bc[:], lhsT=sel2_sb[0:1, 128:256],
                rhs=rr[1][0:1, c0:c1],
                start=False, stop=True)
            bci = pRr.tile([128, 512], F32, tag="bci", name=f"bci{t}_{c0}")
            nc.vector.reciprocal_approx_fast(bci[:, 0:nw], bc[:])
            nc.vector.tensor_tensor(
                ob[:, c0:c1], ob[:, c0:c1], bci[:, 0:nw], MUL)
            nc.sync.dma_start(out_d[t, :, c0:c1], ob[:, c0:c1])

        def emit_attn(t, warm=(), pre=()):
            # warm: projection emitters woven in as dense PE filler so the
            # HAM clock gate never sees an idle window. pre: deferred
            # normalization chunks of the previous q-tile. This tile's
            # first two norm chunks run at i=7/11, a few steps after the
            # denominators they need, so the PE queue never stalls on the
            # ACT extraction chain.
            LOOK = 2
            warm = dict(warm)
            ob = pOut.tile([128, NJ * 256], F32, tag="out", name=f"ob{t}")
            rr = [pRr.tile([1, NJ * 256], F32R, tag="rr0", name=f"rr0_{t}"),
                  pRr.tile([1, NJ * 256], F32R, tag="rr1", name=f"rr1_{t}")]
            prs = {}
            for i in range(H + LOOK):
                if i < H:
                    prs[i] = emit_scores(t, i)
                if i == 1:
                    for fn in pre:
                        fn()
                for fn in warm.pop(i, ()):
                    fn()
                if i == 7:
                    emit_norm_ck(t, ob, rr, 0, 512)
                if i == 11:
                    emit_norm_ck(t, ob, rr, 512, 1024)
                if i >= LOOK:
                    emit_pv(t, i - LOOK, prs.pop(i - LOOK), ob, rr)
            for i in sorted(warm):
                for fn in warm[i]:
                    fn()
            return ob, rr

        # ---- schedule: early slices first so attention overlaps
        # projections; remaining projection work is woven into the attention
        # pipeline as PE filler, and each tile's normalization is deferred
        # behind the next dense block.
        emit_qproj(0)
        emit_kproj(0)
        emit_kproj(1)
        wv_t = []
        for k in range(KB):
            w = pW.tile([128, HID], BF16, tag="w")
            nc.gpsimd.dma_start(w[:], wv_d[k * 128:(k + 1) * 128, :])
            wv_t.append(w)
        for st in range(6):
            emit_vproj(st)
        # q-projections for tiles 2/3 and kproj(2) ride just-in-time inside
        # the attention phases: each chain lands in the PE queue ahead of
        # the first head that reads it, keeping the PE dense to the end
        ob0, rr0 = emit_attn(0, warm={
            3: [lambda: emit_vproj(6)], 5: [lambda: emit_vproj(7)],
            9: [lambda: emit_kproj_m(2, 0)], 13: [lambda: emit_kproj_m(2, 1)]})
        ob1, rr1 = emit_attn(1, warm={
            1: [lambda: emit_qproj_tm(2, 0)], 3: [lambda: emit_qproj_tm(2, 1)],
            5: [lambda: emit_qproj_tm(2, 2)], 7: [lambda: emit_qproj_tm(2, 3)],
            9: [lambda: emit_qproj_tm(2, 4)],
            13: [lambda: emit_qproj_tm(2, 5), lambda: emit_kproj_m(2, 2)],
            14: [lambda: emit_vproj(8), lambda: emit_vproj(9)]},
            pre=(lambda: emit_norm_ck(0, ob0, rr0, 1024, 1536),))
        ob2, rr2 = emit_attn(2, warm={
            1: [lambda: emit_qproj_tm(3, 0)], 3: [lambda: emit_kproj_m(2, 3)],
            5: [lambda: emit_kproj_m(2, 4)], 7: [lambda: emit_qproj_tm(3, 1)],
            9: [lambda: emit_kproj_m(2, 5)], 13: [lambda: emit_vproj(10)]},
            pre=(lambda: emit_norm_ck(1, ob1, rr1, 1024, 1536),))
        ob3, rr3 = emit_attn(3, warm={
            3: [lambda: emit_qproj_tm(3, 2)], 5: [lambda: emit_qproj_tm(3, 3)],
            7: [lambda: emit_qproj_tm(3, 4)], 9: [lambda: emit_qproj_tm(3, 5)]},
            pre=(lambda: emit_vproj(11),
                 lambda: emit_norm_ck(2, ob2, rr2, 1024, 1536)))
        emit_norm_ck(3, ob3, rr3, 1024, 1280)
        emit_norm_ck(3, ob3, rr3, 1280, 1536)

    nc.compile()
    return nc


_NC = None


def _get_nc():
    global _NC
    if _NC is None:
        _NC = _build()
    return _NC


def _bf16(x):
    return np.ascontiguousarray(x.astype(ml_dtypes.bfloat16))


def _prepare_in_maps(hidden_states, Wq, bq, Wk, bk, Wv, bv):
    hidden_states = np.asarray(hidden_states, dtype=np.float32)
    Wq = np.asarray(Wq, dtype=np.float32)
    Wk = np.asarray(Wk, dtype=np.float32)
    Wv = np.asarray(Wv, dtype=np.float32)
    bq = np.asarray(bq, dtype=np.float32)
    bk = np.asarray(bk, dtype=np.float32)
    bv = np.asarray(bv, dtype=np.float32)

    scale = 1.0 / np.sqrt(D).astype(np.float32)
    hT = np.ascontiguousarray(hidden_states.reshape(S, HID).T)  # [768, 8192]
    wq_b = _bf16(Wq * scale)
    wk_b = _bf16(Wk)
    wv_b = _bf16(Wv)
    biasqk = np.concatenate(
        [(bq * scale).reshape(KB, 128).T, bk.reshape(KB, 128).T], axis=1)
    biasqk = np.ascontiguousarray(biasqk, dtype=np.float32)
    bvrow = _bf16(bv.reshape(1, HID))
    sel2 = (np.arange(128)[None, :] // 64 == np.arange(2)[:, None]).reshape(1, 256)
    sel2 = np.ascontiguousarray(sel2.astype(np.float32))

    in_maps = []
    for c in range(NCORES):
        lo, hi = c * SL - W, c * SL + SL + W
        padl, padr = max(0, -lo), max(0, hi - S)
        hT_c = np.zeros((HID, EXT), dtype=np.float32)
        hT_c[:, padl:EXT - padr] = hT[:, lo + padl:hi - padr]
        pv = np.zeros(EXT, dtype=np.float32)
        pv[padl:EXT - padr] = 1.0
        biaspvt = np.concatenate([biasqk, pv.reshape(NST, 128).T], axis=1)
        pvbv = np.concatenate([_bf16(pv.reshape(1, EXT)), bvrow], axis=1)
        in_maps.append(dict(
            sel2=sel2,
            hT=_bf16(hT_c),
            wq=wq_b, wk=wk_b, wv=wv_b,
            biaspvt=np.ascontiguousarray(biaspvt, dtype=np.float32),
            pvbv=np.ascontiguousarray(pvbv),
        ))
    return in_maps


def kernel(hidden_states, Wq, bq, Wk, bk, Wv, bv):
    nc = _get_nc()
    in_maps = _prepare_in_maps(hidden_states, Wq, bq, Wk, bk, Wv, bv)
    res = run_bass_kernel_spmd(nc, in_maps, list(range(NCORES)))
    out = np.empty((NCORES, SL, HID), dtype=np.float32)
    for c in range(NCORES):
        raw = res.results[c]["out"]              # [NT, 128, 1536]
        blk = raw.reshape(NT, 2, 64, NJ, 256)    # [t, hrow, d, hcol, q]
        # head h = hcol*2 + hrow, ctx[t*256+q, h, d]
        out[c] = blk.transpose(0, 4, 3, 1, 2).reshape(SL, HID)
    return out.reshape(1, S, HID)


# revision 42
# speedup vs baseline: 1.0088x; 1.0088x over previous
"""Longformer sliding-window self-attention on 8 Trainium2 NeuronCores.

Problem: hidden [1, 8192, 768] -> QKV projections (768x768 each) ->
12-head sliding-window attention (one-sided window 256) -> ctx [1, 8192, 768].

Sharding: sequence-parallel across 8 cores. Each core owns 1024 query
positions and recomputes K/V projections over its 1024+2*256 halo-extended
slice (host passes the transposed, zero-padded hidden slice per core).

v2 (this file): all matmuls bf16 (hidden/W/q/k/v/probs), which halves
weight-load (LDWEIGHTS) time and input DMA vs the fp32r baseline and keeps
the PE stream dense so the HAM clock gate stays at 8/8. Sequence-boundary
masking is folded into the v' validity column (col 64 per head) and the
zeroed v rows, so only the band-geometry mask remains: a single static
[128, 1024] bf16 mask applied to the 4 of 6 key blocks that intersect the
band edge (score columns are ordered (j0,j1,j4,j5,j2,j3) so one DVE
multiply covers them). Softmax denominators are extracted from PV row 64
by ACT Copy into a [2, 1536] row pair per q-tile and inverted with one
reciprocal_approx_fast, replacing 48 single-partition DVE reciprocals.

Per-core device program:
  - qT [768,1024], kT [768,1536] feature-major bf16 projections.
  - v' [1536, 12*65] bf16 with per-head validity column (softmax
    denominator row); bias+padding via a K=1 matmul against the
    position-validity row.
  - Per (256-query tile x head): scores^T [768k, 256q] in PSUM via 6
    K=64 bf16 matmuls; ACT exp -> bf16 probs; band-mask multiply on the
    first 1024 cols; 6 accumulating bf16 PV matmuls -> ctx' [65, 256].
  - Normalize: ACT-copied denominators -> reciprocal_approx_fast ->
    K=2 selector matmul broadcast -> DVE multiply -> DMA out.
"""
import numpy as np
from contextlib import ExitStack

import ml_dtypes

import concourse.bass as bass
import concourse.bacc as bacc
import concourse.mybir as mybir
from concourse.tile import TileContext
from concourse.bass_utils import run_bass_kernel_spmd

F32 = mybir.dt.float32
F32R = mybir.dt.float32r
BF16 = mybir.dt.bfloat16

NCORES = 8
S, HID, H, D, W = 8192, 768, 12, 64, 256
SL = S // NCORES            # 1024 queries per core
EXT = SL + 2 * W            # 1536 extended positions (with halo)
KB = HID // 128             # 6 feature blocks
NT = SL // 256              # 4 query tiles of 256
NJ = 6                      # key tiles of 128 per query tile
NST = EXT // 128            # 12 sequence tiles for v'
EXPF = mybir.ActivationFunctionType.Exp
COPYF = mybir.ActivationFunctionType.Copy
MUL = mybir.AluOpType.mult
# Compact 1280-col score layout. Key block j0 only matters for the first
# 128 queries of a 256-query tile (band: c <= p < 128) and j5 only for the
# last 128 (c >= p + 128 >= 128), so their valid halves share slot 0:
#   slot0 = [j0 @ q 0:128 | j5 @ q 128:256], slot1 = j1, slot2 = j4,
#   slot3 = j2, slot4 = j3.  Slots 0-2 carry the band mask; 3-4 are fully
#   in-band.  (q0, nq, col) per key block j:
SLOT = {0: (0, 128, 0), 5: (128, 128, 128),
        1: (0, 256, 256), 4: (0, 256, 512),
        2: (0, 256, 768), 3: (0, 256, 1024)}
NSC = 1280                  # score/prob columns per (tile, head)


def _build():
    nc = bacc.Bacc(
        "TRN2",
        target_bir_lowering=False,
        debug=False,
        num_devices=NCORES,
    )
    hT_d = nc.declare_dram_parameter("hT", [HID, EXT], BF16, isOutput=False)
    wq_d = nc.declare_dram_parameter("wq", [HID, HID], BF16, isOutput=False)
    wk_d = nc.declare_dram_parameter("wk", [HID, HID], BF16, isOutput=False)
    wv_d = nc.declare_dram_parameter("wv", [HID, HID], BF16, isOutput=False)
    # biasqk cols 0:12, position-validity (transposed) cols 12:24
    bp_d = nc.declare_dram_parameter("biaspvt", [128, 2 * KB + NST], F32, isOutput=False)
    # validity row 0:EXT, v-bias row EXT:EXT+HID
    pvbv_d = nc.declare_dram_parameter("pvbv", [1, EXT + HID], BF16, isOutput=False)
    sel2_d = nc.declare_dram_parameter("sel2", [1, 256], F32R, isOutput=False)
    out_d = nc.declare_dram_parameter("out", [NT, 128, NJ * 256], F32, isOutput=True)

    with ExitStack() as ctx:
        tc = ctx.enter_context(TileContext(nc))
        pH = ctx.enter_context(tc.tile_pool(name="h", bufs=1))
        pW = ctx.enter_context(tc.tile_pool(name="w", bufs=18))
        pQ = ctx.enter_context(tc.tile_pool(name="q", bufs=1))
        pK = ctx.enter_context(tc.tile_pool(name="k", bufs=1))
        pV = ctx.enter_context(tc.tile_pool(name="v", bufs=1))
        pProb = ctx.enter_context(tc.tile_pool(name="prob", bufs=3))
        pMask = ctx.enter_context(tc.tile_pool(name="mask", bufs=1))
        pOut = ctx.enter_context(tc.tile_pool(name="outp", bufs=2))
        pRr = ctx.enter_context(tc.tile_pool(name="rr", bufs=2))
        pMisc = ctx.enter_context(tc.tile_pool(name="misc", bufs=1))
        pSc = ctx.enter_context(tc.tile_pool(name="scps", bufs=2, space="PSUM"))
        pPs = ctx.enter_context(tc.tile_pool(name="ps", bufs=2, space="PSUM"))

        # ---- weight & hidden DMAs first so projections start ASAP, spread
        # over the three hwdge queues (sync: wq,wk; gpsimd: hT even + wv;
        # scalar: consts + hT odd).  Small constants are batched into two
        # parameters since each DMA costs ~0.6us of queue time regardless
        # of size.
        wq_t = []
        h_t = [None] * KB
        for k in range(KB):
            w = pW.tile([128, HID], BF16, tag="w")
            nc.sync.dma_start(w[:], wq_d[k * 128:(k + 1) * 128, :])
            wq_t.append(w)
            ht = pH.tile([128, EXT], BF16, tag=f"h{k}")
            eng = nc.gpsimd if k % 2 == 0 else nc.scalar
            eng.dma_start(ht[:], hT_d[k * 128:(k + 1) * 128, :])
            h_t[k] = ht
        wk_t = []
        for k in range(KB):
            w = pW.tile([128, HID], BF16, tag="w")
            nc.sync.dma_start(w[:], wk_d[k * 128:(k + 1) * 128, :])
            wk_t.append(w)
        bp_sb = pMisc.tile([128, 2 * KB + NST], F32, tag="biaspvt")
        nc.scalar.dma_start(bp_sb[:], bp_d[:])
        bias_sb = bp_sb[:, 0:2 * KB]
        pvt_sb = bp_sb[:, 2 * KB:]
        pvbv_sb = pMisc.tile([1, EXT + HID], BF16, tag="pvbv")
        nc.scalar.dma_start(pvbv_sb[:], pvbv_d[:])
        pvrow_sb = pvbv_sb[:, 0:EXT]
        bvrow_sb = pvbv_sb[:, EXT:]
        sel2_sb = pMisc.tile([1, 256], F32R, tag="sel2")
        nc.scalar.dma_start(sel2_sb[:], sel2_d[:])

        # static band mask for slots 0-2 of the compact layout:
        #   slot0 lo (j0 @ q 0:128):   valid iff p - c >= 0
        #   slot0 hi (j5 @ q 128:256): valid iff c - p >= 0 (local c)
        #   slot1    (j1 @ q 0:256):   valid iff p + 128 - c >= 0
        #   slot2    (j4 @ q 0:256):   valid iff c - p >= 0
        mk = pMask.tile([128, 3 * 256], BF16, tag="mask", name="mask")
        nc.gpsimd.memset(mk[:], 1.0)
        nc.gpsimd.affine_select(
            out=mk[:, 0:128], in_=mk[:, 0:128],
            compare_op=mybir.AluOpType.is_ge,
            fill=0.0, base=0, pattern=[[-1, 128]], channel_multiplier=1)
        nc.gpsimd.affine_select(
            out=mk[:, 128:256], in_=mk[:, 128:256],
            compare_op=mybir.AluOpType.is_ge,
            fill=0.0, base=0, pattern=[[1, 128]], channel_multiplier=-1)
        nc.gpsimd.affine_select(
            out=mk[:, 256:512], in_=mk[:, 256:512],
            compare_op=mybir.AluOpType.is_ge,
            fill=0.0, base=128, pattern=[[-1, 256]], channel_multiplier=1)
        nc.gpsimd.affine_select(
            out=mk[:, 512:768], in_=mk[:, 512:768],
            compare_op=mybir.AluOpType.is_ge,
            fill=0.0, base=0, pattern=[[1, 256]], channel_multiplier=-1)

        qT_t = [pQ.tile([128, SL], BF16, tag=f"q{m}", name=f"qT{m}") for m in range(KB)]
        kT_t = [pK.tile([128, EXT], BF16, tag=f"k{m}", name=f"kT{m}") for m in range(KB)]
        v_t = [pV.tile([128, H * 65], BF16, tag=f"v{st}", name=f"vp{st}") for st in range(NST)]

        def emit_qproj(c2):
            e0 = W + c2 * 512
            for m in range(KB):
                ps = pPs.tile([128, 512], F32, tag="ps")
                for k in range(KB):
                    nc.tensor.matmul(
                        ps[:], lhsT=wq_t[k][:, m * 128:(m + 1) * 128],
                        rhs=h_t[k][:, e0:e0 + 512],
                        start=(k == 0), stop=(k == KB - 1))
                nc.vector.tensor_scalar_add(
                    qT_t[m][:, c2 * 512:(c2 + 1) * 512], ps[:],
                    bias_sb[:, m:m + 1])

        def emit_qproj_tm(t, m):
            # N=256 q-projection chain for one (q-tile, row-block): rides
            # just-in-time inside the attention of earlier tiles
            e0 = W + t * 256
            ps = pPs.tile([128, 256], F32, tag="ps")
            for k in range(KB):
                nc.tensor.matmul(
                    ps[:], lhsT=wq_t[k][:, m * 128:(m + 1) * 128],
                    rhs=h_t[k][:, e0:e0 + 256],
                    start=(k == 0), stop=(k == KB - 1))
            nc.vector.tensor_scalar_add(
                qT_t[m][:, t * 256:(t + 1) * 256], ps[:],
                bias_sb[:, m:m + 1])

        def emit_kproj_m(c, m):
            ps = pPs.tile([128, 512], F32, tag="ps")
            for k in range(KB):
                nc.tensor.matmul(
                    ps[:], lhsT=wk_t[k][:, m * 128:(m + 1) * 128],
                    rhs=h_t[k][:, c * 512:(c + 1) * 512],
                    start=(k == 0), stop=(k == KB - 1))
            nc.vector.tensor_scalar_add(
                kT_t[m][:, c * 512:(c + 1) * 512], ps[:],
                bias_sb[:, KB + m:KB + m + 1])

        def emit_kproj(c):
            for m in range(KB):
                emit_kproj_m(c, m)

        def emit_vproj(st):
            vt = v_t[st]
            vv = vt[:].rearrange("p (h x) -> p h x", x=65)
            # validity column doubles as the softmax-denominator selector:
            # invalid (zero-padded) key rows contribute to neither ctx nor
            # the denominator, which subsumes the sequence-boundary mask
            dcol = vv[:, :, 64:65]
            pvv = pvt_sb[:, st:st + 1].rearrange("p (a b) -> p a b", a=1)
            _, pvb = bass.broadcast_tensor_aps(dcol, pvv)
            nc.vector.tensor_copy(dcol, pvb)
            for (f0, nf) in ((0, 512), (512, 256)):
                ps = pPs.tile([128, nf], F32, tag="ps")
                for k in range(KB):
                    nc.tensor.matmul(
                        ps[:], lhsT=h_t[k][:, st * 128:(st + 1) * 128],
                        rhs=wv_t[k][:, f0:f0 + nf],
                        start=(k == 0), stop=False)
                nc.tensor.matmul(
                    ps[:], lhsT=pvrow_sb[0:1, st * 128:(st + 1) * 128],
                    rhs=bvrow_sb[0:1, f0:f0 + nf], start=False, stop=True)
                nc.scalar.activation(
                    vv[:, f0 // 64:(f0 + nf) // 64, 0:64],
                    ps[:].rearrange("p (h x) -> p h x", x=64), COPYF)

        def emit_scores(t, h):
            kb, po = h // 2, (h % 2) * 64
            sc = pSc.tile([128, NSC], F32, tag="sc")
            for j in (0, 5, 1, 4, 2, 3):
                q0, nq, cs = SLOT[j]
                k0 = t * 256 + j * 128
                nc.tensor.matmul(
                    sc[:, cs:cs + nq],
                    lhsT=kT_t[kb][po:po + 64, k0:k0 + 128],
                    rhs=qT_t[kb][po:po + 64, t * 256 + q0:t * 256 + q0 + nq],
                    start=True, stop=True)
            pr = pProb.tile([128, NSC], BF16, tag="pr")
            nc.scalar.activation(pr[:], sc[:], EXPF)
            nc.vector.tensor_mul(pr[:, 0:768], pr[:, 0:768], mk[:])
            return pr

        def emit_pv(t, h, prm, ob, rr):
            # j1 (full width) opens the accumulation group — a start on a
            # half-width block would zero the other half's columns
            cx = pPs.tile([65, 256], F32, tag="ps")
            for j in (1, 0, 5, 4, 2, 3):
                q0, nq, cs = SLOT[j]
                nc.tensor.matmul(
                    cx[:, q0:q0 + nq],
                    lhsT=v_t[2 * t + j][:, h * 65:(h + 1) * 65],
                    rhs=prm[:, cs:cs + nq],
                    start=(j == 1), stop=(j == 3),
                    skip_group_check=True)
            # stash unnormalized ctx' and the denominator row; normalization
            # is batched per q-tile so the PE stream stays dense
            c0 = (h // 2) * 256
            nc.vector.tensor_copy(
                ob[(h % 2) * 64:(h % 2) * 64 + 64, c0:c0 + 256], cx[0:64, :])
            nc.vector.tensor_copy(
                rr[h % 2][0:1, c0:c0 + 256], cx[64:65, :])

        def emit_norm_ck(t, ob, rr, c0, c1):
            # broadcast the RAW denominators across partitions with K=1
            # selector matmuls, then invert the wide [128, c1-c0] tile
            # (c1-c0 elems/lane: fast) and multiply — avoids
            # single-partition reciprocals entirely. One chunk at a time so
            # normalization + output DMA pipeline behind the attention.
            nw = c1 - c0
            bc = pPs.tile([128, nw], F32, tag="ps")
            nc.tensor.matmul(
                bc[:], lhsT=sel2_sb[0:1, 0:128],
                rhs=rr[0][0:1, c0:c1],
                start=True, stop=False)
            nc.tensor.matmul(
                bc[:], lhsT=sel2_sb[0:1, 128:256],
                rhs=rr[1][0:1, c0:c1],
                start=False, stop=True)
            bci = pRr.tile([128, 512], F32, tag="bci", name=f"bci{t}_{c0}")
            nc.vector.reciprocal_approx_fast(bci[:, 0:nw], bc[:])
            nc.vector.tensor_tensor(
                ob[:, c0:c1], ob[:, c0:c1], bci[:, 0:nw], MUL)
            nc.sync.dma_start(out_d[t, :, c0:c1], ob[:, c0:c1])

        def emit_attn(t, warm=(), pre=()):
            # warm: projection emitters woven in as dense PE filler so the
            # HAM clock gate never sees an idle window. pre: deferred
            # normalization chunks of the previous q-tile. This tile's
            # first two norm chunks run at i=7/11, a few steps after the
            # denominators they need, so the PE queue never stalls on the
            # ACT extraction chain.
            LOOK = 2
            warm = dict(warm)
            ob = pOut.tile([128, NJ * 256], F32, tag="out", name=f"ob{t}")
            rr = [pRr.tile([1, NJ * 256], F32R, tag="rr0", name=f"rr0_{t}"),
                  pRr.tile([1, NJ * 256], F32R, tag="rr1", name=f"rr1_{t}")]
            prs = {}
            for i in range(H + LOOK):
                if i < H:
                    prs[i] = emit_scores(t, i)
                if i == 1:
                    for fn in pre:
                        fn()
                for fn in warm.pop(i, ()):
                    fn()
                if i == 7:
                    emit_norm_ck(t, ob, rr, 0, 512)
                if i == 11:
                    emit_norm_ck(t, ob, rr, 512, 1024)
                if i >= LOOK:
                    emit_pv(t, i - LOOK, prs.pop(i - LOOK), ob, rr)
            for i in sorted(warm):
                for fn in warm[i]:
                    fn()
            return ob, rr

        # ---- schedule: early slices first so attention overlaps
        # projections; remaining projection work is woven into the attention
        # pipeline as PE filler, and each tile's normalization is deferred
        # behind the next dense block.
        emit_qproj(0)
        for m in range(KB):
            emit_qproj_tm(2, m)
        emit_kproj(0)
        emit_kproj(1)
        wv_t = []
        for k in range(KB):
            w = pW.tile([128, HID], BF16, tag="w")
            nc.gpsimd.dma_start(w[:], wv_d[k * 128:(k + 1) * 128, :])
            wv_t.append(w)
        for st in range(6):
            emit_vproj(st)
        # tile-3 q-projection and kproj(2) ride just-in-time inside the
        # attention phases: each chain lands in the PE queue ahead of the
        # first head that reads it, keeping the PE dense to the very end
        ob0, rr0 = emit_attn(0, warm={
            3: [lambda: emit_vproj(6)], 5: [lambda: emit_vproj(7)],
            9: [lambda: emit_kproj_m(2, 0)], 13: [lambda: emit_kproj_m(2, 1)]})
        ob1, rr1 = emit_attn(1, warm={
            1: [lambda: emit_kproj_m(2, 2)], 3: [lambda: emit_qproj_tm(3, 0)],
            5: [lambda: emit_qproj_tm(3, 1)],
            13: [lambda: emit_vproj(8)], 14: [lambda: emit_vproj(9)]},
            pre=(lambda: emit_norm_ck(0, ob0, rr0, 1024, 1536),))
        ob2, rr2 = emit_attn(2, warm={
            1: [lambda: emit_kproj_m(2, 3)], 3: [lambda: emit_kproj_m(2, 4)],
            5: [lambda: emit_qproj_tm(3, 2)], 9: [lambda: emit_kproj_m(2, 5)],
            13: [lambda: emit_vproj(10)]},
            pre=(lambda: emit_norm_ck(1, ob1, rr1, 1024, 1536),))
        ob3, rr3 = emit_attn(3, warm={
            3: [lambda: emit_qproj_tm(3, 3)], 5: [lambda: emit_qproj_tm(3, 4)],
            7: [lambda: emit_qproj_tm(3, 5)]},
            pre=(lambda: emit_vproj(11),
                 lambda: emit_norm_ck(2, ob2, rr2, 1024, 1536)))
        emit_norm_ck(3, ob3, rr3, 1024, 1280)
        emit_norm_ck(3, ob3, rr3, 1280, 1536)

    nc.compile()
    return nc


_NC = None


def _get_nc():
    global _NC
    if _NC is None:
        _NC = _build()
    return _NC


def _bf16(x):
    return np.ascontiguousarray(x.astype(ml_dtypes.bfloat16))


def _prepare_in_maps(hidden_states, Wq, bq, Wk, bk, Wv, bv):
    hidden_states = np.asarray(hidden_states, dtype=np.float32)
    Wq = np.asarray(Wq, dtype=np.float32)
    Wk = np.asarray(Wk, dtype=np.float32)
    Wv = np.asarray(Wv, dtype=np.float32)
    bq = np.asarray(bq, dtype=np.float32)
    bk = np.asarray(bk, dtype=np.float32)
    bv = np.asarray(bv, dtype=np.float32)

    scale = 1.0 / np.sqrt(D).astype(np.float32)
    hT = np.ascontiguousarray(hidden_states.reshape(S, HID).T)  # [768, 8192]
    wq_b = _bf16(Wq * scale)
    wk_b = _bf16(Wk)
    wv_b = _bf16(Wv)
    biasqk = np.concatenate(
        [(bq * scale).reshape(KB, 128).T, bk.reshape(KB, 128).T], axis=1)
    biasqk = np.ascontiguousarray(biasqk, dtype=np.float32)
    bvrow = _bf16(bv.reshape(1, HID))
    sel2 = (np.arange(128)[None, :] // 64 == np.arange(2)[:, None]).reshape(1, 256)
    sel2 = np.ascontiguousarray(sel2.astype(np.float32))

    in_maps = []
    for c in range(NCORES):
        lo, hi = c * SL - W, c * SL + SL + W
        padl, padr = max(0, -lo), max(0, hi - S)
        hT_c = np.zeros((HID, EXT), dtype=np.float32)
        hT_c[:, padl:EXT - padr] = hT[:, lo + padl:hi - padr]
        pv = np.zeros(EXT, dtype=np.float32)
        pv[padl:EXT - padr] = 1.0
        biaspvt = np.concatenate([biasqk, pv.reshape(NST, 128).T], axis=1)
        pvbv = np.concatenate([_bf16(pv.reshape(1, EXT)), bvrow], axis=1)
        in_maps.append(dict(
            sel2=sel2,
            hT=_bf16(hT_c),
            wq=wq_b, wk=wk_b, wv=wv_b,
            biaspvt=np.ascontiguousarray(biaspvt, dtype=np.float32),
            pvbv=np.ascontiguousarray(pvbv),
        ))
    return in_maps


def kernel(hidden_states, Wq, bq, Wk, bk, Wv, bv):
    nc = _get_nc()
    in_maps = _prepare_in_maps(hidden_states, Wq, bq, Wk, bk, Wv, bv)
    res = run_bass_kernel_spmd(nc, in_maps, list(range(NCORES)))
    out = np.empty((NCORES, SL, HID), dtype=np.float32)
    for c in range(NCORES):
        raw = res.results[c]["out"]              # [NT, 128, 1536]
        blk = raw.reshape(NT, 2, 64, NJ, 256)    # [t, hrow, d, hcol, q]
        # head h = hcol*2 + hrow, ctx[t*256+q, h, d]
        out[c] = blk.transpose(0, 4, 3, 1, 2).reshape(SL, HID)
    return out.reshape(1, S, HID)
